# revision 2
# baseline (speedup 1.0000x reference)
"""Trainium2 Bass kernel for nn_AttnDecoderRNN (attention-LSTM decoder step).

Self-contained: hardcodes shapes from the problem spec.

Math (from the reference):
  weights = softmax(attn, axis=-1) over a size-1 axis == exactly ones(S,1),
  so enc_ctx = column-sum of encoder_outputs and Wattn/battn are dead.
  x  = concat([emb[_input], enc_ctx])            # (3072,)
  pre_g = x @ Wg_x.T + hidden @ Wg_h.T + bg_x + bg_h   for g in {f,i,o,g}
  f,i,o = sigmoid(...), g = tanh(...)
  c_new = i*g + context*f ; h_new = o*tanh(c_new)
  logits = h_new @ Wout.T + bout

Sharding over 8 cores (one NEFF, SPMD):
  - gates: contraction-dim sharded. Core k owns x-slice
    [embed 128k:128k+128 | enc_ctx 256k:256k+256 | hidden 256k:256k+256]
    and computes partial pre-activations for all 2048 gate outputs of all
    4 gates; one 32 KiB AllReduce combines partials; every core then
    computes c_new/h_new redundantly (tiny).
  - enc_ctx: column-sharded; core k column-sums encoder_outputs[:, 256k:+256]
    on the PE (ones-vector matmul, PSUM-accumulated).
  - logits: vocab-row sharded. Core k streams Wout rows [6272k : 6272k+6353]
    (overlapping views so all cores run the identical instruction stream)
    and computes its logits slice with one fused dot-product per 128 rows
    (scalar_tensor_tensor with accum_out), h_new broadcast across partitions
    via a K=1 PE outer product into PSUM.

Per-core j-layout for 2048-vectors: j = p*16 + a  (partition p, col a),
so [2048] DRAM <-> [128,16] SBUF DMAs are contiguous 64B runs.
"""
import os
import sys
import time

sys.path.insert(0, "/opt/trn_rl_repo")

import numpy as np

H = 2048
E = 1024
V = 50257
S = 2048
NCORES = 8
ESL = E // NCORES        # 128  embed slice
HSL = H // NCORES        # 256  hidden / enc_ctx slice
XSL = ESL + 2 * HSL      # 640  per-core contraction width
VSL_STEP = 6272          # 49*128: stride between core vocab shards
VSL = 6353               # 49*128 + 81: rows each core loads (overlapping)
NVT = 50                 # vocab tiles per core (49 full + 1 of 81 rows)
VLAST = VSL - 49 * 128   # 81

USE_BF16 = os.environ.get("KBF16", "0") == "1"

_CACHE = {}


def _build(n_rep=1):
    import concourse.bass as bass
    import concourse.tile as tile
    from concourse import bacc, mybir
    from contextlib import ExitStack

    dt = mybir.dt
    f32 = dt.float32
    wdt = dt.bfloat16 if USE_BF16 else dt.float32
    AOP = mybir.AluOpType
    AF = mybir.ActivationFunctionType

    nc = bacc.Bacc("TRN2", target_bir_lowering=False, debug=False,
                   num_devices=NCORES)

    # ---- DRAM I/O ----
    enc_s = nc.dram_tensor("enc_s", [S, HSL], f32, kind="ExternalInput")
    wg = [nc.dram_tensor(f"w{g}", [H, XSL], wdt, kind="ExternalInput")
          for g in range(4)]  # order: f, i, o, g
    wout = nc.dram_tensor("wout", [VSL, H], wdt, kind="ExternalInput")
    bout_p = nc.dram_tensor("bout_p", [128, NVT], f32, kind="ExternalInput")
    bias_p = nc.dram_tensor("bias_p", [128, 64], f32, kind="ExternalInput")
    ctx_p = nc.dram_tensor("ctx_p", [128, 16], f32, kind="ExternalInput")
    emb_k = nc.dram_tensor("emb_k", [1, ESL], f32, kind="ExternalInput")
    hid_k = nc.dram_tensor("hid_k", [1, HSL], f32, kind="ExternalInput")

    logits_o = nc.dram_tensor("logits_o", [n_rep, 128, NVT], f32,
                              kind="ExternalOutput")
    h_o = nc.dram_tensor("h_o", [n_rep, 128, 16], f32, kind="ExternalOutput")
    c_o = nc.dram_tensor("c_o", [n_rep, 128, 16], f32, kind="ExternalOutput")

    with tile.TileContext(nc) as tc:
        for rep in range(n_rep):
            with ExitStack() as ctx:
                sb = ctx.enter_context(
                    tc.tile_pool(name=f"sb{rep}", bufs=1))
                gpool = ctx.enter_context(
                    tc.tile_pool(name=f"gp{rep}", bufs=3))
                wpool = ctx.enter_context(
                    tc.tile_pool(name=f"wp{rep}", bufs=12))
                ps = ctx.enter_context(
                    tc.tile_pool(name=f"ps{rep}", bufs=1, space="PSUM"))
                dram = ctx.enter_context(
                    tc.tile_pool(name=f"dr{rep}", bufs=1, space="DRAM"))

                # constants
                ones_r = sb.tile([1, 128], f32)      # K=1 outer-product lhsT
                nc.vector.memset(ones_r[:], 1.0)
                ones_c = sb.tile([128, 1], f32)      # colsum lhsT
                nc.vector.memset(ones_c[:], 1.0)

                # ---- phase 1: enc_ctx slice = colsum(encoder_outputs[:, k-slice])
                enc_t = sb.tile([128, 16, HSL], f32)
                nc.sync.dma_start(
                    enc_t[:],
                    enc_s.ap().rearrange("(c p) f -> p c f", p=128))
                xh_row = sb.tile([1, XSL], f32)
                nc.scalar.dma_start(xh_row[0:1, 0:ESL], emb_k.ap()[:])
                nc.scalar.dma_start(xh_row[0:1, ESL + HSL:XSL], hid_k.ap()[:])
                enc_ps = ps.tile([1, HSL], f32)
                for c in range(16):
                    nc.tensor.matmul(enc_ps[:], lhsT=ones_c[:],
                                     rhs=enc_t[:, c, :],
                                     start=(c == 0), stop=(c == 15))
                nc.vector.tensor_copy(xh_row[0:1, ESL:ESL + HSL], enc_ps[:])

                # broadcast xh across partitions via K=1 outer product
                xh_ps = ps.tile([128, XSL], f32)
                nc.tensor.matmul(xh_ps[:, 0:512], lhsT=ones_r[:],
                                 rhs=xh_row[0:1, 0:512], start=True, stop=True)
                nc.tensor.matmul(xh_ps[:, 512:XSL], lhsT=ones_r[:],
                                 rhs=xh_row[0:1, 512:XSL], start=True, stop=True)

                # ---- phase 2: gate partials ----
                scratch = sb.tile([128, H], wdt)
                gates_sb = sb.tile([128, 4, 16], f32)
                xh_in1 = xh_ps
                for g in range(4):
                    wga = wg[g].ap().rearrange("(p a) c -> p a c", a=16)
                    for half in range(2):
                        gt = gpool.tile([128, 8, XSL], wdt)
                        nc.sync.dma_start(gt[:], wga[:, half * 8:half * 8 + 8, :])
                        for a in range(8):
                            nc.vector.scalar_tensor_tensor(
                                out=scratch[:, 0:XSL],
                                in0=gt[:, a, :], scalar=1.0, in1=xh_in1[:],
                                op0=AOP.mult, op1=AOP.mult,
                                accum_out=gates_sb[:, g, half * 8 + a:half * 8 + a + 1])

                # ---- phase 3: AllReduce partials ----
                b_in = dram.tile([128, 64], f32)
                b_out = dram.tile([128, 64], f32)
                nc.scalar.dma_start(b_in[:], gates_sb[:].rearrange("p g a -> p (g a)"))
                nc.gpsimd.collective_compute(
                    "AllReduce", AOP.add,
                    replica_groups=[list(range(NCORES))],
                    ins=[b_in.opt()], outs=[b_out.opt()])
                gsum = sb.tile([128, 64], f32)
                nc.scalar.dma_start(gsum[:], b_out[:])

                # ---- phase 4: nonlinearities + cell update (replicated) ----
                bias_t = sb.tile([128, 64], f32)
                nc.scalar.dma_start(bias_t[:], bias_p.ap()[:])
                ctx_t = sb.tile([128, 16], f32)
                nc.scalar.dma_start(ctx_t[:], ctx_p.ap()[:])
                nc.vector.tensor_tensor(out=gsum[:], in0=gsum[:], in1=bias_t[:],
                                        op=AOP.add)
                acts = sb.tile([128, 64], f32)
                nc.scalar.activation(out=acts[:, 0:48], in_=gsum[:, 0:48],
                                     func=AF.Sigmoid)
                nc.scalar.activation(out=acts[:, 48:64], in_=gsum[:, 48:64],
                                     func=AF.Tanh)
                av = acts[:].rearrange("p (g a) -> p g a", g=4)
                t1 = sb.tile([128, 16], f32)
                c_t = sb.tile([128, 16], f32)
                h_t = sb.tile([128, 16], f32)
                nc.vector.tensor_tensor(out=t1[:], in0=av[:, 1, :], in1=av[:, 3, :],
                                        op=AOP.mult)
                nc.vector.tensor_tensor(out=c_t[:], in0=av[:, 0, :], in1=ctx_t[:],
                                        op=AOP.mult)
                nc.vector.tensor_tensor(out=c_t[:], in0=c_t[:], in1=t1[:],
                                        op=AOP.add)
                nc.scalar.dma_start(c_o.ap()[rep], c_t[:])
                tc_t = sb.tile([128, 16], f32)
                nc.scalar.activation(out=tc_t[:], in_=c_t[:], func=AF.Tanh)
                nc.vector.tensor_tensor(out=h_t[:], in0=av[:, 2, :], in1=tc_t[:],
                                        op=AOP.mult)
                nc.scalar.dma_start(h_o.ap()[rep], h_t[:])

                # gather h (j = p*16+a) into one row, broadcast via PE
                h_row = sb.tile([1, H], f32)
                nc.scalar.dma_start(h_row[0:1, :], h_t[:])
                h_ps = ps.tile([128, H], f32)
                for q in range(4):
                    nc.tensor.matmul(h_ps[:, q * 512:(q + 1) * 512],
                                     lhsT=ones_r[:],
                                     rhs=h_row[0:1, q * 512:(q + 1) * 512],
                                     start=True, stop=True)
                if USE_BF16:
                    h_in1 = sb.tile([128, H], dt.bfloat16)
                    nc.vector.tensor_copy(h_in1[:], h_ps[:])
                else:
                    h_in1 = h_ps

                # ---- phase 5: logits = Wout_shard @ h + bout ----
                lacc = sb.tile([128, NVT], f32)
                for t in range(NVT):
                    rows = 128 if t < NVT - 1 else VLAST
                    wt = wpool.tile([128, H], wdt)
                    nc.sync.dma_start(wt[:rows, :],
                                      wout.ap()[t * 128:t * 128 + rows, :])
                    nc.vector.scalar_tensor_tensor(
                        out=scratch[:rows, :], in0=wt[:rows, :], scalar=1.0,
                        in1=h_in1[:rows, :], op0=AOP.mult, op1=AOP.mult,
                        accum_out=lacc[:rows, t:t + 1])
                bout_t = sb.tile([128, NVT], f32)
                nc.scalar.dma_start(bout_t[:], bout_p.ap()[:])
                nc.vector.tensor_tensor(out=lacc[:], in0=lacc[:], in1=bout_t[:],
                                        op=AOP.add)
                nc.scalar.dma_start(logits_o.ap()[rep], lacc[:])

    nc.compile()
    return nc


def _get_nc(n_rep=1):
    key = (n_rep, USE_BF16)
    if key not in _CACHE:
        _CACHE[key] = _build(n_rep)
    return _CACHE[key]


def _prep_inputs(_input, hidden, context, encoder_outputs, emb,
                 Wf_x, bf_x, Wf_h, bf_h, Wi_x, bi_x, Wi_h, bi_h,
                 Wg_x, bg_x, Wg_h, bg_h, Wo_x, bo_x, Wo_h, bo_h,
                 Wattn, battn, Wout, bout):
    f4 = np.float32
    wnp = np.float32
    if USE_BF16:
        import ml_dtypes
        wnp = ml_dtypes.bfloat16

    idx = int(np.asarray(_input).reshape(-1)[0])
    e_row = np.asarray(emb[idx], dtype=f4).reshape(-1)            # (1024,)
    hid = np.asarray(hidden, dtype=f4).reshape(-1)                # (2048,)
    ctx = np.asarray(context, dtype=f4).reshape(-1)               # (2048,)
    ctx_p = np.ascontiguousarray(ctx.reshape(128, 16))

    gates_x = [Wf_x, Wi_x, Wo_x, Wg_x]   # order f, i, o, g
    gates_h = [Wf_h, Wi_h, Wo_h, Wg_h]
    bias = np.stack([
        np.asarray(bf_x) + np.asarray(bf_h),
        np.asarray(bi_x) + np.asarray(bi_h),
        np.asarray(bo_x) + np.asarray(bo_h),
        np.asarray(bg_x) + np.asarray(bg_h),
    ]).astype(f4)                                                  # (4, 2048)
    bias_p = np.ascontiguousarray(
        bias.reshape(4, 128, 16).transpose(1, 0, 2).reshape(128, 64))

    Wout = np.asarray(Wout)
    bout = np.asarray(bout, dtype=f4)
    enc = np.asarray(encoder_outputs, dtype=f4)

    in_maps = []
    for k in range(NCORES):
        m = {}
        m["enc_s"] = np.ascontiguousarray(enc[:, k * HSL:(k + 1) * HSL])
        for g in range(4):
            wx = np.asarray(gates_x[g])
            wh = np.asarray(gates_h[g])
            m[f"w{g}"] = np.concatenate(
                [wx[:, k * ESL:(k + 1) * ESL],
                 wx[:, E + k * HSL:E + (k + 1) * HSL],
                 wh[:, k * HSL:(k + 1) * HSL]], axis=1).astype(wnp)
        r0 = k * VSL_STEP if k < NCORES - 1 else V - VSL
        m["wout"] = np.ascontiguousarray(Wout[r0:r0 + VSL]).astype(wnp)
        bo = np.zeros(NVT * 128, f4)
        bo[:VSL] = bout[r0:r0 + VSL]
        m["bout_p"] = np.ascontiguousarray(bo.reshape(NVT, 128).T)
        m["bias_p"] = bias_p
        m["ctx_p"] = ctx_p
        m["emb_k"] = np.ascontiguousarray(e_row[k * ESL:(k + 1) * ESL]).reshape(1, -1)
        m["hid_k"] = np.ascontiguousarray(hid[k * HSL:(k + 1) * HSL]).reshape(1, -1)
        in_maps.append(m)
    return in_maps


def _assemble(results, rep=0):
    logits = np.empty(V, np.float32)
    for k in range(NCORES):
        r0 = k * VSL_STEP if k < NCORES - 1 else V - VSL
        n = VSL_STEP if k < NCORES - 1 else VSL
        la = results[k]["logits_o"][rep]                # (128, 50)
        flat = la.T.reshape(-1)                         # v = t*128 + p
        logits[r0:r0 + n] = flat[:n]
    h_new = results[0]["h_o"][rep].reshape(1, H).astype(np.float32)
    c_new = results[0]["c_o"][rep].reshape(1, H).astype(np.float32)
    weights = np.ones((S, 1), np.float32)
    return logits.reshape(1, V), h_new, c_new, weights


def run_on_hw(in_maps, n_rep=1):
    from concourse import bass_utils
    nc = _get_nc(n_rep)
    t0 = time.time()
    res = bass_utils.run_bass_kernel_spmd(
        nc, in_maps, core_ids=list(range(NCORES)))
    wall = time.time() - t0
    return res.results, wall


def kernel(**inputs):
    in_maps = _prep_inputs(**inputs)
    results, _ = run_on_hw(in_maps, n_rep=1)
    return _assemble(results)


if __name__ == "__main__":
    # quick self-drive with random data
    rng = np.random.default_rng(0)
    inputs = {
        "_input": np.array([123]), "hidden": rng.standard_normal((1, H)).astype(np.float32) * 0.1,
        "context": rng.standard_normal((1, H)).astype(np.float32) * 0.1,
        "encoder_outputs": rng.standard_normal((S, H)).astype(np.float32) * 0.1,
        "emb": rng.standard_normal((V, E)).astype(np.float32) * 0.02,
    }
    for g in "figo":
        inputs[f"W{g}_x"] = rng.standard_normal((H, H + E)).astype(np.float32) * 0.02
        inputs[f"b{g}_x"] = rng.standard_normal(H).astype(np.float32) * 0.02
        inputs[f"W{g}_h"] = rng.standard_normal((H, H)).astype(np.float32) * 0.02
        inputs[f"b{g}_h"] = rng.standard_normal(H).astype(np.float32) * 0.02
    inputs["Wattn"] = rng.standard_normal((1, 2 * H)).astype(np.float32)
    inputs["battn"] = rng.standard_normal(1).astype(np.float32)
    inputs["Wout"] = rng.standard_normal((V, H)).astype(np.float32) * 0.02
    inputs["bout"] = rng.standard_normal(V).astype(np.float32) * 0.02
    out = kernel(**inputs)
    print([o.shape for o in out])


# revision 15
# speedup vs baseline: 1.6219x; 1.6219x over previous
"""Trainium2 Bass kernel for nn_AttnDecoderRNN (attention-LSTM decoder step).

Self-contained: hardcodes shapes from the problem spec.

Math (from the reference):
  weights = softmax(attn, axis=-1) over a size-1 axis == exactly ones(S,1),
  so enc_ctx = column-sum of encoder_outputs and Wattn/battn are dead.
  x  = concat([emb[_input], enc_ctx])            # (3072,)
  pre_g = x @ Wg_x.T + hidden @ Wg_h.T + bg_x + bg_h   for g in {f,i,o,g}
  f,i,o = sigmoid(...), g = tanh(...)
  c_new = i*g + context*f ; h_new = o*tanh(c_new)
  logits = h_new @ Wout.T + bout

Sharding over 8 cores (one NEFF, SPMD; per-core inputs carry the shards):
  - gates: contraction-dim sharded. Core k owns the x-slice
    [embed 128k:+128 | enc_ctx 256k:+256 | hidden 256k:+256] (640 wide) and
    computes partial pre-activations for all 4x2048 gate outputs on the PE
    (host pre-transposes gate weight slices to [640, 2048] fp16); one 32 KiB
    AllReduce combines partials; every core then computes c_new/h_new
    redundantly (tiny).
  - enc_ctx: column-sharded; core k column-sums encoder_outputs[:, 256k:+256]
    on the PE (ones-vector matmul, PSUM-accumulated).
  - logits: vocab-row sharded (~6353 rows/core, overlapping views so all
    cores run the identical instruction stream). Split between PE (host
    pre-transposed fp16 [2048, PE_T*128], 16*PE_T small matmuls) and DVE
    (native-layout fp16 rows, fused dot-product scalar_tensor_tensor with
    accum_out) to balance engine occupancy; DMA is the bottleneck.

Per-core j-layout for 2048-vectors: j = p*16 + a  (partition p, col a),
so [2048] DRAM <-> [128,16] SBUF DMAs are contiguous 64B runs.

KPREC env: fp16 (default, v2) | f32 (exact, v1) | wbf16 / bf16 (v1 variants)
"""
import os
import sys
import time

sys.path.insert(0, "/opt/trn_rl_repo")

import numpy as np

H = 2048
E = 1024
V = 50257
S = 2048
NCORES = 8
ESL = E // NCORES        # 128  embed slice
HSL = H // NCORES        # 256  hidden / enc_ctx slice
XSL = ESL + 2 * HSL      # 640  per-core contraction width
VSL_STEP = 6272          # 49*128: stride between core vocab shards
VSL = 6353               # 49*128 + 81: rows each core loads (overlapping)
NVT = 50                 # vocab tiles per core (49 full + 1 of 81 rows)
VLAST = VSL - 49 * 128   # 81
PE_T = 32                # vocab tiles computed on the PE (rest on DVE)
DVE_T0 = PE_T            # first DVE tile index

KPREC = os.environ.get("KPREC", "fp16")
WOUT_BF16 = KPREC in ("wbf16", "bf16")
GATES_BF16 = KPREC == "bf16"

_CACHE = {}


# --------------------------------------------------------------------------
# v2 builder: fp16 weights, gates + PE_T/50 of Wout on PE, rest on DVE
# --------------------------------------------------------------------------
def _build_v2(n_rep=1):
    import concourse.bass as bass
    import concourse.tile as tile
    from concourse import bacc, mybir
    from contextlib import ExitStack

    dt = mybir.dt
    f32 = dt.float32
    f16 = dt.float16
    AOP = mybir.AluOpType
    AF = mybir.ActivationFunctionType

    nc = bacc.Bacc("TRN2", target_bir_lowering=False, debug=False,
                   num_devices=NCORES)

    # ---- DRAM I/O ----
    enc_s = nc.dram_tensor("enc_s", [S, HSL], f32, kind="ExternalInput")
    wgt = [nc.dram_tensor(f"wgt{g}", [XSL, H], f16, kind="ExternalInput")
           for g in range(4)]  # transposed gate slices; order f, i, o, g
    woutT = nc.dram_tensor("woutT", [H, PE_T * 128], f16, kind="ExternalInput")
    wout_d = nc.dram_tensor("wout_d", [VSL - PE_T * 128, H], f16,
                            kind="ExternalInput")
    bout_p = nc.dram_tensor("bout_p", [128, NVT], f32, kind="ExternalInput")
    bias_p = nc.dram_tensor("bias_p", [128, 64], f32, kind="ExternalInput")
    ctx_p = nc.dram_tensor("ctx_p", [128, 16], f32, kind="ExternalInput")
    emb_k = nc.dram_tensor("emb_k", [128, 1], f16, kind="ExternalInput")
    hid_k = nc.dram_tensor("hid_k", [128, 2], f16, kind="ExternalInput")

    logits_o = nc.dram_tensor("logits_o", [n_rep, 128, NVT], f32,
                              kind="ExternalOutput")
    h_o = nc.dram_tensor("h_o", [n_rep, 128, 16], f32, kind="ExternalOutput")
    c_o = nc.dram_tensor("c_o", [n_rep, 128, 16], f32, kind="ExternalOutput")
    DBG = os.environ.get("KDBG", "0") == "1"
    if DBG:
        dbg1 = nc.dram_tensor("dbg1", [1, 4 * H], f32, kind="ExternalOutput")
        dbg2 = nc.dram_tensor("dbg2", [128, 64], f32, kind="ExternalOutput")
        dbg3 = nc.dram_tensor("dbg3", [128, 5], f32, kind="ExternalOutput")
        dbg4 = nc.dram_tensor("dbg4", [128, 16], f32, kind="ExternalOutput")

    with tile.TileContext(nc) as tc:
        for rep in range(n_rep):
            with ExitStack() as ctx:
                sb = ctx.enter_context(tc.tile_pool(name=f"sb{rep}", bufs=1))
                gpool = ctx.enter_context(tc.tile_pool(name=f"gp{rep}", bufs=2))
                wpool = ctx.enter_context(tc.tile_pool(name=f"wp{rep}", bufs=6))
                dpool = ctx.enter_context(tc.tile_pool(name=f"dp{rep}", bufs=10))
                dram = ctx.enter_context(
                    tc.tile_pool(name=f"dr{rep}", bufs=1, space="DRAM"))

                ones_c = sb.tile([128, 1], f32)
                nc.vector.memset(ones_c[:], 1.0)
                ones_r = sb.tile([1, 128], f32)
                nc.vector.memset(ones_r[:], 1.0)

                # small input tiles up front (scalar HWDGE ring)
                bias_t = sb.tile([128, 64], f32)
                nc.scalar.dma_start(bias_t[:], bias_p.ap()[:])
                ctx_t = sb.tile([128, 16], f32)
                nc.scalar.dma_start(ctx_t[:], ctx_p.ap()[:])
                bout_t = sb.tile([128, NVT], f32)
                nc.scalar.dma_start(bout_t[:], bout_p.ap()[:])

                # ---- phase 1: enc_ctx slice directly in column layout ----
                with ExitStack() as ps_ctx:
                    psG = ps_ctx.enter_context(
                        tc.tile_pool(name=f"psG{rep}", bufs=2, space="PSUM"))
                    enc_t = sb.tile([128, 16, HSL], f32)
                    nc.sync.dma_start(
                        enc_t[:], enc_s.ap().rearrange("(c p) f -> p c f", p=128))
                    # xh_col[p, c] = xh[c*128 + p]; chunks: 0=emb, 1-2=enc, 3-4=hid
                    xh_col = sb.tile([128, 5], f16)
                    nc.sync.dma_start(xh_col[:, 0:1], emb_k.ap()[:])
                    nc.sync.dma_start(xh_col[:, 3:5], hid_k.ap()[:])
                    enc_col = psG.tile([128, 2], f32, name="enc_col", tag="psum_g")
                    for j in range(2):
                        for c in range(16):
                            nc.tensor.matmul(enc_col[:, j:j + 1],
                                             lhsT=enc_t[:, c, j * 128:(j + 1) * 128],
                                             rhs=ones_c[:],
                                             start=(c == 0), stop=(c == 15),
                                             skip_group_check=True)
                    nc.vector.tensor_copy(xh_col[:, 1:3], enc_col[:])

                    # ---- phase 2: gate partials on PE ----
                    sbrow = sb.tile([1, 4 * H], f32)
                    for g in range(4):
                        gt = gpool.tile([128, 5, H], f16)
                        nc.sync.dma_start(
                            gt[:], wgt[g].ap().rearrange("(c p) j -> p c j", p=128))
                        psum_g = psG.tile([1, H], f32, name="psum_g", tag="psum_g")
                        for c in range(5):
                            for n in range(4):
                                nc.tensor.matmul(
                                    psum_g[0:1, n * 512:(n + 1) * 512],
                                    lhsT=xh_col[:, c:c + 1],
                                    rhs=gt[:, c, n * 512:(n + 1) * 512],
                                    start=(c == 0), stop=(c == 4),
                                    skip_group_check=True)
                        nc.vector.tensor_copy(sbrow[0:1, g * H:(g + 1) * H],
                                              psum_g[:])

                    if DBG and rep == 0:
                        nc.scalar.dma_start(dbg1.ap()[:], sbrow[:])
                        xc32d = sb.tile([128, 5], f32)
                        nc.vector.tensor_copy(xc32d[:], xh_col[:])
                        nc.scalar.dma_start(dbg3.ap()[:], xc32d[:])
                        ec32d = sb.tile([128, 2], f32)
                        nc.vector.tensor_copy(ec32d[:], enc_col[:])
                        nc.scalar.dma_start(dbg4.ap()[0:128, 0:2], ec32d[:])
                    # ---- phase 3: AllReduce partials (gpsimd ring, same engine
                    # as the collective so RAW ordering is same-queue FIFO) ----
                    b_in = dram.tile([1, 4 * H], f32)
                    b_out = dram.tile([1, 4 * H], f32)
                    nc.gpsimd.dma_start(b_in[:], sbrow[:])
                    nc.gpsimd.collective_compute(
                        "AllReduce", AOP.add,
                        replica_groups=[list(range(NCORES))],
                        ins=[b_in.opt()], outs=[b_out.opt()])
                    gsum = sb.tile([128, 4, 16], f32)
                    nc.gpsimd.dma_start(
                        gsum[:], b_out[:].rearrange("o (g p a) -> (o p) g a",
                                                    g=4, p=128))

                # ---- phase 4: nonlinearities + cell update (replicated) ----
                gs = gsum[:].rearrange("p g a -> p (g a)")
                nc.vector.tensor_tensor(out=gs, in0=gs, in1=bias_t[:], op=AOP.add)
                acts = sb.tile([128, 64], f32)
                nc.scalar.activation(out=acts[:, 0:48], in_=gs[:, 0:48],
                                     func=AF.Sigmoid)
                nc.scalar.activation(out=acts[:, 48:64], in_=gs[:, 48:64],
                                     func=AF.Tanh)
                av = acts[:].rearrange("p (g a) -> p g a", g=4)
                t1 = sb.tile([128, 16], f32)
                c_t = sb.tile([128, 16], f32)
                h_t = sb.tile([128, 16], f32)
                nc.vector.tensor_tensor(out=t1[:], in0=av[:, 1, :],
                                        in1=av[:, 3, :], op=AOP.mult)
                nc.vector.tensor_tensor(out=c_t[:], in0=av[:, 0, :],
                                        in1=ctx_t[:], op=AOP.mult)
                nc.vector.tensor_tensor(out=c_t[:], in0=c_t[:], in1=t1[:],
                                        op=AOP.add)
                nc.scalar.dma_start(c_o.ap()[rep], c_t[:])
                tc_t = sb.tile([128, 16], f32)
                nc.scalar.activation(out=tc_t[:], in_=c_t[:], func=AF.Tanh)
                nc.vector.tensor_tensor(out=h_t[:], in0=av[:, 2, :],
                                        in1=tc_t[:], op=AOP.mult)
                nc.scalar.dma_start(h_o.ap()[rep], h_t[:])

                # h_t16: fp16 cast of h in natural [128,16] layout (PE rhs)
                h_t16 = sb.tile([128, 16], f16)
                nc.vector.tensor_copy(h_t16[:], h_t[:])
                # h_row (j order) for the DVE-share broadcast
                h_row = sb.tile([1, H], f32)
                nc.gpsimd.dma_start(h_row[0:1, :], h_t[:])

                with ExitStack() as ps_ctx:
                    psB = ps_ctx.enter_context(
                        tc.tile_pool(name=f"psB{rep}", bufs=1, space="PSUM"))
                    h_ps = psB.tile([128, H], f32)
                    for q in range(4):
                        nc.tensor.matmul(h_ps[:, q * 512:(q + 1) * 512],
                                         lhsT=ones_r[:],
                                         rhs=h_row[0:1, q * 512:(q + 1) * 512],
                                         start=True, stop=True)
                    h_bf = sb.tile([128, H], f16)
                    nc.vector.tensor_copy(h_bf[:], h_ps[:])

                    # ---- phase 5: logits ----
                    lacc = sb.tile([128, NVT], f32)
                    lacc_pe = sb.tile([128, PE_T], f32)
                    nc.vector.memset(lacc_pe[:], 0.0)
                    # PE share: tiles [0, PE_T); one-shot matmuls per (c, t)
                    # (start=True clears the whole bank's has_written bits, so
                    # interleaved open groups in one bank are not allowed);
                    # accumulate across c-chunks on DVE in SBUF.
                    wTa = woutT.ap().rearrange("(c p) v -> p c v", p=128)
                    for c in range(16):
                        wtc = wpool.tile([128, PE_T * 128], f16)
                        nc.sync.dma_start(wtc[:], wTa[:, c, :])
                        psum_lc = psB.tile([128, PE_T], f32,
                                           name="psum_lc", tag="psum_lc", bufs=2)
                        for t in range(PE_T):
                            nc.tensor.matmul(
                                psum_lc[:, t:t + 1],
                                lhsT=wtc[:, t * 128:(t + 1) * 128],
                                rhs=h_t16[:, c:c + 1],
                                start=True, stop=True,
                                skip_group_check=True)
                        nc.vector.tensor_tensor(out=lacc_pe[:], in0=lacc_pe[:],
                                                in1=psum_lc[:],
                                                op=AOP.add)
                    nc.vector.tensor_copy(lacc[:, 0:PE_T], lacc_pe[:])
                    # DVE share: tiles [PE_T, NVT)
                    scratch = sb.tile([128, H], f16)
                    for t in range(DVE_T0, NVT):
                        rows = 128 if t < NVT - 1 else VLAST
                        r0 = (t - DVE_T0) * 128
                        wt = dpool.tile([128, H], f16)
                        nc.sync.dma_start(wt[:rows, :],
                                          wout_d.ap()[r0:r0 + rows, :])
                        nc.vector.scalar_tensor_tensor(
                            out=scratch[:rows, :], in0=wt[:rows, :], scalar=1.0,
                            in1=h_bf[:rows, :], op0=AOP.mult, op1=AOP.mult,
                            accum_out=lacc[:rows, t:t + 1])
                    nc.vector.tensor_tensor(out=lacc[:], in0=lacc[:],
                                            in1=bout_t[:], op=AOP.add)
                    nc.scalar.dma_start(logits_o.ap()[rep], lacc[:])

    nc.compile()
    return nc


# --------------------------------------------------------------------------
# v1 builder: STT everywhere (f32 exact, or bf16 Wout) — fallback path
# --------------------------------------------------------------------------
def _build_v1(n_rep=1):
    import concourse.bass as bass
    import concourse.tile as tile
    from concourse import bacc, mybir
    from contextlib import ExitStack

    dt = mybir.dt
    f32 = dt.float32
    wdt_g = dt.bfloat16 if GATES_BF16 else dt.float32
    wdt_o = dt.bfloat16 if WOUT_BF16 else dt.float32
    AOP = mybir.AluOpType
    AF = mybir.ActivationFunctionType

    nc = bacc.Bacc("TRN2", target_bir_lowering=False, debug=False,
                   num_devices=NCORES)

    enc_s = nc.dram_tensor("enc_s", [S, HSL], f32, kind="ExternalInput")
    wg = [nc.dram_tensor(f"w{g}", [H, XSL], wdt_g, kind="ExternalInput")
          for g in range(4)]  # order: f, i, o, g
    wout = nc.dram_tensor("wout", [VSL, H], wdt_o, kind="ExternalInput")
    bout_p = nc.dram_tensor("bout_p", [128, NVT], f32, kind="ExternalInput")
    bias_p = nc.dram_tensor("bias_p", [128, 64], f32, kind="ExternalInput")
    ctx_p = nc.dram_tensor("ctx_p", [128, 16], f32, kind="ExternalInput")
    emb_k = nc.dram_tensor("emb_k", [1, ESL], f32, kind="ExternalInput")
    hid_k = nc.dram_tensor("hid_k", [1, HSL], f32, kind="ExternalInput")

    logits_o = nc.dram_tensor("logits_o", [n_rep, 128, NVT], f32,
                              kind="ExternalOutput")
    h_o = nc.dram_tensor("h_o", [n_rep, 128, 16], f32, kind="ExternalOutput")
    c_o = nc.dram_tensor("c_o", [n_rep, 128, 16], f32, kind="ExternalOutput")

    with tile.TileContext(nc) as tc:
        for rep in range(n_rep):
            with ExitStack() as ctx:
                sb = ctx.enter_context(tc.tile_pool(name=f"sb{rep}", bufs=1))
                gpool = ctx.enter_context(tc.tile_pool(name=f"gp{rep}", bufs=3))
                wpool = ctx.enter_context(tc.tile_pool(name=f"wp{rep}", bufs=12))
                ps = ctx.enter_context(
                    tc.tile_pool(name=f"ps{rep}", bufs=1, space="PSUM"))
                dram = ctx.enter_context(
                    tc.tile_pool(name=f"dr{rep}", bufs=1, space="DRAM"))

                ones_r = sb.tile([1, 128], f32)
                nc.vector.memset(ones_r[:], 1.0)
                ones_c = sb.tile([128, 1], f32)
                nc.vector.memset(ones_c[:], 1.0)

                enc_t = sb.tile([128, 16, HSL], f32)
                nc.sync.dma_start(
                    enc_t[:], enc_s.ap().rearrange("(c p) f -> p c f", p=128))
                xh_row = sb.tile([1, XSL], f32)
                nc.scalar.dma_start(xh_row[0:1, 0:ESL], emb_k.ap()[:])
                nc.scalar.dma_start(xh_row[0:1, ESL + HSL:XSL], hid_k.ap()[:])
                enc_ps = ps.tile([1, HSL], f32)
                for c in range(16):
                    nc.tensor.matmul(enc_ps[:], lhsT=ones_c[:],
                                     rhs=enc_t[:, c, :],
                                     start=(c == 0), stop=(c == 15))
                nc.vector.tensor_copy(xh_row[0:1, ESL:ESL + HSL], enc_ps[:])

                xh_ps = ps.tile([128, XSL], f32)
                nc.tensor.matmul(xh_ps[:, 0:512], lhsT=ones_r[:],
                                 rhs=xh_row[0:1, 0:512], start=True, stop=True)
                nc.tensor.matmul(xh_ps[:, 512:XSL], lhsT=ones_r[:],
                                 rhs=xh_row[0:1, 512:XSL], start=True, stop=True)

                scratch = sb.tile([128, H], wdt_o)
                scratch_g = sb.tile([128, XSL], wdt_g)
                gates_sb = sb.tile([128, 4, 16], f32)
                if GATES_BF16:
                    xh_in1 = sb.tile([128, XSL], dt.bfloat16, name="xh_bf")
                    nc.vector.tensor_copy(xh_in1[:], xh_ps[:])
                else:
                    xh_in1 = xh_ps
                for g in range(4):
                    wga = wg[g].ap().rearrange("(p a) c -> p a c", a=16)
                    for half in range(2):
                        gt = gpool.tile([128, 8, XSL], wdt_g)
                        nc.sync.dma_start(gt[:], wga[:, half * 8:half * 8 + 8, :])
                        for a in range(8):
                            nc.vector.scalar_tensor_tensor(
                                out=scratch_g[:],
                                in0=gt[:, a, :], scalar=1.0, in1=xh_in1[:],
                                op0=AOP.mult, op1=AOP.mult,
                                accum_out=gates_sb[:, g, half * 8 + a:half * 8 + a + 1])

                b_in = dram.tile([128, 64], f32)
                b_out = dram.tile([128, 64], f32)
                nc.scalar.dma_start(b_in[:], gates_sb[:].rearrange("p g a -> p (g a)"))
                nc.gpsimd.collective_compute(
                    "AllReduce", AOP.add,
                    replica_groups=[list(range(NCORES))],
                    ins=[b_in.opt()], outs=[b_out.opt()])
                gsum = sb.tile([128, 64], f32)
                nc.scalar.dma_start(gsum[:], b_out[:])

                bias_t = sb.tile([128, 64], f32)
                nc.scalar.dma_start(bias_t[:], bias_p.ap()[:])
                ctx_t = sb.tile([128, 16], f32)
                nc.scalar.dma_start(ctx_t[:], ctx_p.ap()[:])
                nc.vector.tensor_tensor(out=gsum[:], in0=gsum[:], in1=bias_t[:],
                                        op=AOP.add)
                acts = sb.tile([128, 64], f32)
                nc.scalar.activation(out=acts[:, 0:48], in_=gsum[:, 0:48],
                                     func=AF.Sigmoid)
                nc.scalar.activation(out=acts[:, 48:64], in_=gsum[:, 48:64],
                                     func=AF.Tanh)
                av = acts[:].rearrange("p (g a) -> p g a", g=4)
                t1 = sb.tile([128, 16], f32)
                c_t = sb.tile([128, 16], f32)
                h_t = sb.tile([128, 16], f32)
                nc.vector.tensor_tensor(out=t1[:], in0=av[:, 1, :], in1=av[:, 3, :],
                                        op=AOP.mult)
                nc.vector.tensor_tensor(out=c_t[:], in0=av[:, 0, :], in1=ctx_t[:],
                                        op=AOP.mult)
                nc.vector.tensor_tensor(out=c_t[:], in0=c_t[:], in1=t1[:],
                                        op=AOP.add)
                nc.scalar.dma_start(c_o.ap()[rep], c_t[:])
                tc_t = sb.tile([128, 16], f32)
                nc.scalar.activation(out=tc_t[:], in_=c_t[:], func=AF.Tanh)
                nc.vector.tensor_tensor(out=h_t[:], in0=av[:, 2, :], in1=tc_t[:],
                                        op=AOP.mult)
                nc.scalar.dma_start(h_o.ap()[rep], h_t[:])

                h_row = sb.tile([1, H], f32)
                nc.scalar.dma_start(h_row[0:1, :], h_t[:])
                h_ps = ps.tile([128, H], f32)
                for q in range(4):
                    nc.tensor.matmul(h_ps[:, q * 512:(q + 1) * 512],
                                     lhsT=ones_r[:],
                                     rhs=h_row[0:1, q * 512:(q + 1) * 512],
                                     start=True, stop=True)
                if WOUT_BF16:
                    h_in1 = sb.tile([128, H], dt.bfloat16, name="h_bf")
                    nc.vector.tensor_copy(h_in1[:], h_ps[:])
                else:
                    h_in1 = h_ps

                lacc = sb.tile([128, NVT], f32)
                for t in range(NVT):
                    rows = 128 if t < NVT - 1 else VLAST
                    wt = wpool.tile([128, H], wdt_o)
                    nc.sync.dma_start(wt[:rows, :],
                                      wout.ap()[t * 128:t * 128 + rows, :])
                    nc.vector.scalar_tensor_tensor(
                        out=scratch[:rows, :], in0=wt[:rows, :], scalar=1.0,
                        in1=h_in1[:rows, :], op0=AOP.mult, op1=AOP.mult,
                        accum_out=lacc[:rows, t:t + 1])
                bout_t = sb.tile([128, NVT], f32)
                nc.scalar.dma_start(bout_t[:], bout_p.ap()[:])
                nc.vector.tensor_tensor(out=lacc[:], in0=lacc[:], in1=bout_t[:],
                                        op=AOP.add)
                nc.scalar.dma_start(logits_o.ap()[rep], lacc[:])

    nc.compile()
    return nc


def _get_nc(n_rep=1):
    key = (n_rep, KPREC)
    if key not in _CACHE:
        if KPREC == "fp16":
            _CACHE[key] = _build_v2(n_rep)
        else:
            _CACHE[key] = _build_v1(n_rep)
    return _CACHE[key]


def _prep_inputs(_input, hidden, context, encoder_outputs, emb,
                 Wf_x, bf_x, Wf_h, bf_h, Wi_x, bi_x, Wi_h, bi_h,
                 Wg_x, bg_x, Wg_h, bg_h, Wo_x, bo_x, Wo_h, bo_h,
                 Wattn, battn, Wout, bout):
    f4 = np.float32
    v2 = KPREC == "fp16"
    wnp_g = np.float32
    wnp_o = np.float32
    if WOUT_BF16 or GATES_BF16:
        import ml_dtypes
        if GATES_BF16:
            wnp_g = ml_dtypes.bfloat16
        if WOUT_BF16:
            wnp_o = ml_dtypes.bfloat16

    idx = int(np.asarray(_input).reshape(-1)[0])
    e_row = np.asarray(emb[idx], dtype=f4).reshape(-1)            # (1024,)
    hid = np.asarray(hidden, dtype=f4).reshape(-1)                # (2048,)
    ctx = np.asarray(context, dtype=f4).reshape(-1)               # (2048,)
    ctx_p = np.ascontiguousarray(ctx.reshape(128, 16))

    gates_x = [Wf_x, Wi_x, Wo_x, Wg_x]   # order f, i, o, g
    gates_h = [Wf_h, Wi_h, Wo_h, Wg_h]
    bias = np.stack([
        np.asarray(bf_x) + np.asarray(bf_h),
        np.asarray(bi_x) + np.asarray(bi_h),
        np.asarray(bo_x) + np.asarray(bo_h),
        np.asarray(bg_x) + np.asarray(bg_h),
    ]).astype(f4)                                                  # (4, 2048)
    bias_p = np.ascontiguousarray(
        bias.reshape(4, 128, 16).transpose(1, 0, 2).reshape(128, 64))

    Wout = np.asarray(Wout)
    bout = np.asarray(bout, dtype=f4)
    enc = np.asarray(encoder_outputs, dtype=f4)

    in_maps = []
    for k in range(NCORES):
        m = {}
        m["enc_s"] = np.ascontiguousarray(enc[:, k * HSL:(k + 1) * HSL])
        for g in range(4):
            wx = np.asarray(gates_x[g])
            wh = np.asarray(gates_h[g])
            wall = np.concatenate(
                [wx[:, k * ESL:(k + 1) * ESL],
                 wx[:, E + k * HSL:E + (k + 1) * HSL],
                 wh[:, k * HSL:(k + 1) * HSL]], axis=1)
            if v2:
                m[f"wgt{g}"] = np.ascontiguousarray(wall.T).astype(np.float16)
            else:
                m[f"w{g}"] = wall.astype(wnp_g)
        r0 = k * VSL_STEP if k < NCORES - 1 else V - VSL
        shard = Wout[r0:r0 + VSL]
        if v2:
            wT = shard[:PE_T * 128].T.astype(np.float16)      # [2048, PE_T*128]
            m["woutT"] = np.ascontiguousarray(
                wT.reshape(128, 16, -1).transpose(1, 0, 2).reshape(H, -1))
            m["wout_d"] = np.ascontiguousarray(
                shard[PE_T * 128:]).astype(np.float16)
        else:
            m["wout"] = np.ascontiguousarray(shard).astype(wnp_o)
        bo = np.zeros(NVT * 128, f4)
        bo[:VSL] = bout[r0:r0 + VSL]
        m["bout_p"] = np.ascontiguousarray(bo.reshape(NVT, 128).T)
        m["bias_p"] = bias_p
        m["ctx_p"] = ctx_p
        if v2:
            m["emb_k"] = np.ascontiguousarray(
                e_row[k * ESL:(k + 1) * ESL].astype(np.float16).reshape(128, 1))
            m["hid_k"] = np.ascontiguousarray(
                hid[k * HSL:(k + 1) * HSL].astype(np.float16)
                .reshape(2, 128).T)
        else:
            m["emb_k"] = np.ascontiguousarray(
                e_row[k * ESL:(k + 1) * ESL]).reshape(1, -1)
            m["hid_k"] = np.ascontiguousarray(
                hid[k * HSL:(k + 1) * HSL]).reshape(1, -1)
        in_maps.append(m)
    return in_maps


def _assemble(results, rep=0):
    logits = np.empty(V, np.float32)
    for k in range(NCORES):
        r0 = k * VSL_STEP if k < NCORES - 1 else V - VSL
        n = VSL_STEP if k < NCORES - 1 else VSL
        la = results[k]["logits_o"][rep]                # (128, 50)
        flat = la.T.reshape(-1)                         # v = t*128 + p
        logits[r0:r0 + n] = flat[:n]
    h_new = results[0]["h_o"][rep].reshape(1, H).astype(np.float32)
    c_new = results[0]["c_o"][rep].reshape(1, H).astype(np.float32)
    weights = np.ones((S, 1), np.float32)
    return logits.reshape(1, V), h_new, c_new, weights


def run_on_hw(in_maps, n_rep=1):
    from concourse import bass_utils
    nc = _get_nc(n_rep)
    t0 = time.time()
    res = bass_utils.run_bass_kernel_spmd(
        nc, in_maps, core_ids=list(range(NCORES)))
    wall = time.time() - t0
    return res.results, wall


def kernel(**inputs):
    in_maps = _prep_inputs(**inputs)
    results, _ = run_on_hw(in_maps, n_rep=1)
    return _assemble(results)
